# revision 21
# baseline (speedup 1.0000x reference)
"""Trainium2 Bass kernel for nn_Attention (sparse_attention).

Math (reference collapsed):
  va[b]    = ht[b] @ Wa_w          (host, tiny)
  ca[b]    = ht[b] . Wa_b          (host, tiny)
  energy   = leaky_relu(hs . va + ca), masked to -1e4 where s >= state_len
  alpha    = softmax(energy)                               (device)
  u[b]     = sum_s alpha[b,s] * hs[b,s,:]                  (device)
  context  = u @ Wc_w.T + Wc_b                             (host, tiny)

Sparsity: positions s >= state_len[b] have alpha exactly 0 (f32 exp
underflow) and never touch hs, so only ceil(state_len/128) s-tiles per
batch are shipped/computed. state_len is known when kernel() runs, so the
per-slot tile counts are baked into the Bass graph (rebuilt per distinct
state_len signature). Batches are assigned to cores in descending-length
groups of 8 so each slot's padding (to the slot max) is minimal.

Device strategy: pure data-parallel, 4 batch-slots/core x 8 cores, no
collectives. hs is shipped twice in fp16 (pre-transposed layout for the
energy matvec, partition-major natural layout for the alpha-weighted sum)
so both matmuls contract over the SBUF partition axis.
"""

import sys

sys.path.insert(0, "/opt/trn_rl_repo")

import numpy as np

import concourse.bass as bass
import concourse.tile as tile
from concourse import mybir
from concourse.bass_utils import run_bass_kernel_spmd
from concourse.vector_clock import ScopedClock

B, S, E = 32, 8192, 256
NCORES = 8
BL = B // NCORES  # batch slots per core
NEG_SLOPE = 0.2
MASK_VAL = -10000.0
AF = mybir.ActivationFunctionType
ALU = mybir.AluOpType

_PATCHED = False


def _patch_tile_drain():
    """Walrus in this env rejects >1 sem-wait on the kernel-exit Drain CTRL.

    Hoist the end-of-kernel waits onto single-wait sync nops instead.
    """
    global _PATCHED
    if _PATCHED:
        return
    _PATCHED = True

    def _drain_and_barrier(self, tick_clock, wait_clock):
        nc = self.nc
        carrier = nc.sync.nop(nofuse=True, hint="tile_exit_wait_carrier")
        wait_clock.add_sem_waits(
            carrier.ins, ScopedClock({None: tick_clock.global_clock})
        )
        si = carrier.ins.sync_info
        waits = list(si.on_wait) if si is not None else []
        if len(waits) > 1:
            carrier.ins.sync_info = mybir.SyncInfo(
                on_wait=[waits[0]], on_update=list(si.on_update)
            )
            for w in waits[1:]:
                n2 = nc.sync.nop(nofuse=True, hint="tile_exit_wait")
                n2.ins.sync_info = mybir.SyncInfo(on_wait=[w], on_update=[])
        nc.sync.drain(fusable=False)
        nc.all_engine_barrier()
        assert self.sems is not None
        popped = nc._tile_sem_poison_stack.pop()
        assert popped is self._sem_poison
        nc.clear_and_free_semaphores(list(self.sems.allocated().values()))
        nc.all_engine_barrier()

    tile.TileContext._drain_and_barrier = _drain_and_barrier


def _split_sync_waits(nc, max_waits=1):
    """Walrus in this env rejects >N sem-waits on a single instruction.

    Hoist excess waits onto same-engine NoOps placed immediately before the
    instruction — the engine sequencer executes in order, so waiting earlier
    on the same engine is semantically identical.
    """
    counter = 0
    for fn in nc.m.functions:
        for blk in fn.blocks:
            insts = list(blk.instructions)
            out = []
            changed = False
            for inst in insts:
                si = inst.sync_info
                waits = list(si.on_wait) if si is not None else []
                if len(waits) > max_waits:
                    keep = waits[:max_waits]
                    for w in waits[max_waits:]:
                        nop = mybir.InstNoOp(name=f"WSPLIT-{counter}")
                        counter += 1
                        nop.engine = inst.engine
                        nop.sync_info = mybir.SyncInfo(on_wait=[w], on_update=[])
                        out.append(nop)
                    inst.sync_info = mybir.SyncInfo(
                        on_wait=keep, on_update=list(si.on_update)
                    )
                    changed = True
                out.append(inst)
            if changed:
                blk.instructions = out


def _build_bass(slot_tiles, reps=1, dma_only=False):
    """slot_tiles: tuple of BL ints — s-tile count for each batch slot."""
    _patch_tile_drain()
    nc = bass.Bass("TRN2", target_bir_lowering=False, num_devices=NCORES)
    f16, f32 = mybir.dt.float16, mybir.dt.float32

    hsT_p, hsn_p, aux_p, alpha_p = [], [], [], []
    for j, nt in enumerate(slot_tiles):
        sj = nt * 128
        hsT_p.append(
            nc.declare_dram_parameter(f"hsT{j}", [128, 2 * sj], f16, isOutput=False)
        )
        # partition-major natural layout: hsn[p, t, :] = hs[t*128+p, :]
        hsn_p.append(
            nc.declare_dram_parameter(f"hsn{j}", [128, nt, E], f16, isOutput=False)
        )
        # aux[:, :nt] = mask cap (+BIG valid / -1e4 invalid), aux[:, nt] = ca
        aux_p.append(
            nc.declare_dram_parameter(f"aux{j}", [128, nt + 1], f32, isOutput=False)
        )
        alpha_p.append(
            nc.declare_dram_parameter(f"alpha{j}", [128, nt], f32, isOutput=True)
        )
    va16 = nc.declare_dram_parameter("va16", [BL, 128, 2], f16, isOutput=False)
    # consts[:, 0:128] = eye(128), consts[:, 128:256] = all-ones
    consts = nc.declare_dram_parameter("consts", [128, 256], f32, isOutput=False)
    u_o = nc.declare_dram_parameter("u", [BL, 1, E], f32, isOutput=True)

    max_nt = max(slot_tiles)
    with tile.TileContext(nc) as tc:
        with (
            tc.tile_pool(name="big", bufs=3) as big,
            tc.tile_pool(name="small", bufs=4) as small,
            tc.tile_pool(name="ones", bufs=1) as onesp,
            tc.tile_pool(name="pse", bufs=3, space="PSUM") as pse,
            tc.tile_pool(name="psu", bufs=2, space="PSUM") as psu,
            tc.tile_pool(name="pss", bufs=3, space="PSUM") as pss,
        ):
            cst = onesp.tile([128, 256], f32)
            nc.sync.dma_start(out=cst[:], in_=consts[:])
            ident = cst[:, 0:128]
            onescol = cst[:, 128:129]
            onesrow = cst[0:1, 128:256]

            def _slot_loads(j):
                nt = slot_tiles[j]
                sj = nt * 128
                # tags sized by max slot so all slots share pool slots
                hsT = big.tile([128, 2 * max_nt * 128], f16, tag="hsT")
                chunks = [(0, nt)] if nt <= 16 else [(0, nt // 2), (nt // 2, nt)]
                for lo, hi in chunks:
                    for h in range(2):
                        nc.sync.dma_start(
                            out=hsT[:, h * sj + lo * 128 : h * sj + hi * 128],
                            in_=hsT_p[j][:, h * sj + lo * 128 : h * sj + hi * 128],
                        )
                hsn = big.tile([128, max_nt, E], f16, tag="hsn")
                for lo, hi in chunks:
                    nc.scalar.dma_start(
                        out=hsn[:, lo:hi, :], in_=hsn_p[j][:, lo:hi, :]
                    )
                va = small.tile([128, 2], f16, tag="va")
                nc.scalar.dma_start(out=va[:], in_=va16[j])
                ax = small.tile([128, max_nt + 1], f32, tag="aux")
                nc.scalar.dma_start(out=ax[:, 0 : nt + 1], in_=aux_p[j][:])
                return hsT, hsn, va, ax

            def _slot_body(j, loaded):
                nt = slot_tiles[j]
                sj = nt * 128
                hsT, hsn, va, ax = loaded

                if dma_only:
                    a32d = small.tile([128, max_nt], f32, tag="a32")
                    nc.vector.tensor_copy(a32d[:, 0:nt], ax[:, 0:nt])
                    nc.sync.dma_start(out=alpha_p[j][:], in_=a32d[:, 0:nt])
                    return

                # ---- energy: e[t*128+p] = sum_e hs[s,e]*va[e]  (PE) ----
                pe = pse.tile([128, max_nt], f32, tag="pe")
                for t in range(nt):
                    for h in range(2):
                        nc.tensor.matmul(
                            out=pe[:, t : t + 1],
                            lhsT=hsT[:, h * sj + t * 128 : h * sj + (t + 1) * 128],
                            rhs=va[:, h : h + 1],
                            start=(h == 0),
                            stop=(h == 1),
                        )

                # ---- epilogue: lrelu(energy + ca) ----
                esb = small.tile([128, max_nt], f32, tag="esb")
                nc.scalar.activation(
                    out=esb[:, 0:nt],
                    in_=pe[:, 0:nt],
                    func=AF.Lrelu,
                    bias=ax[:, nt : nt + 1],
                    scale=1.0,
                    alpha=NEG_SLOPE,
                )
                # ---- mask via min(e, cap), then row-max ----
                em = small.tile([128, max_nt], f32, tag="em")
                nc.vector.tensor_tensor(
                    out=em[:, 0:nt], in0=esb[:, 0:nt], in1=ax[:, 0:nt], op=ALU.min
                )
                mrow = small.tile([128, 1], f32, tag="mrow")
                nc.vector.reduce_max(
                    out=mrow[:], in_=em[:, 0:nt], axis=mybir.AxisListType.X
                )
                # ---- global max: transpose mrow via identity-matmul, reduce,
                #      broadcast back via ones-matmul, negate ----
                psT = pss.tile([1, 128], f32, tag="ps_small")
                nc.tensor.matmul(
                    out=psT[:], lhsT=mrow[:], rhs=ident, start=True, stop=True
                )
                msc = small.tile([1, 1], f32, tag="msc")
                nc.vector.reduce_max(out=msc[:], in_=psT[:], axis=mybir.AxisListType.X)
                psB = pss.tile([128, 1], f32, tag="ps_small")
                nc.tensor.matmul(
                    out=psB[:], lhsT=onesrow, rhs=msc[:], start=True, stop=True
                )
                negm = small.tile([128, 1], f32, tag="negm")
                nc.vector.tensor_scalar_mul(negm[:], psB[:], -1.0)
                # ---- exp (+ row sums) ----
                p16 = small.tile([128, max_nt], f16, tag="p16")
                zrow = small.tile([128, 1], f32, tag="zrow")
                nc.scalar.activation(
                    out=p16[:, 0:nt],
                    in_=em[:, 0:nt],
                    func=AF.Exp,
                    bias=negm[:],
                    scale=1.0,
                    accum_out=zrow[:],
                )
                # ---- global sum via ones-matmul, reciprocal, broadcast ----
                psZ = pss.tile([1, 1], f32, tag="ps_small")
                nc.tensor.matmul(
                    out=psZ[:], lhsT=zrow[:], rhs=onescol, start=True, stop=True
                )
                rz1 = small.tile([1, 1], f32, tag="rz1")
                nc.vector.reciprocal(rz1[:], psZ[:])
                psR = pss.tile([128, 1], f32, tag="ps_small")
                nc.tensor.matmul(
                    out=psR[:], lhsT=onesrow, rhs=rz1[:], start=True, stop=True
                )
                rz = small.tile([128, 1], f32, tag="rz")
                nc.vector.tensor_copy(rz[:], psR[:])

                # ---- alpha output ----
                a32 = small.tile([128, max_nt], f32, tag="a32")
                nc.vector.tensor_scalar_mul(a32[:, 0:nt], p16[:, 0:nt], rz[:])
                nc.sync.dma_start(out=alpha_p[j][:], in_=a32[:, 0:nt])

                # ---- u = sum_s p[s]*hs[s,:] (PE, accumulated), then /z ----
                pu = psu.tile([1, E], f32, tag="pu")
                for t in range(nt):
                    nc.tensor.matmul(
                        out=pu[:],
                        lhsT=p16[:, t : t + 1],
                        rhs=hsn[:, t, :],
                        start=(t == 0),
                        stop=(t == nt - 1),
                    )
                usb = small.tile([1, E], f32, tag="usb")
                nc.scalar.activation(
                    out=usb[:], in_=pu[:], func=AF.Copy, scale=rz[0:1, :]
                )
                nc.sync.dma_start(out=u_o[j], in_=usb[:])

            def _trace_all():
                # loads for slot j+1 trace before slot j's compute so the
                # scheduler prioritizes keeping the DMA queues full
                loaded = {0: _slot_loads(0), 1: _slot_loads(1)}
                for j in range(BL):
                    nxt = j + 2
                    if nxt < BL:
                        loaded[nxt] = _slot_loads(nxt)
                    _slot_body(j, loaded.pop(j))

            if reps == 1:
                _trace_all()
            else:
                with tc.For_i(0, reps, 1):
                    _trace_all()
    _split_sync_waits(nc)
    return nc


_NC_CACHE = {}
last_results = None
last_in_maps = None
last_slot_tiles = None


def kernel(hs, ht, state_len, Wa_w, Wa_b, Wc_w, Wc_b, **run_kwargs):
    global last_results, last_in_maps, last_slot_tiles
    hs = np.asarray(hs, np.float32)
    ht = np.asarray(ht, np.float32)
    state_len = np.asarray(state_len, np.int32)
    Wa_w = np.asarray(Wa_w, np.float32)
    Wa_b = np.asarray(Wa_b, np.float32)
    Wc_w = np.asarray(Wc_w, np.float32)
    Wc_b = np.asarray(Wc_b, np.float32)

    # host-side tiny precomputes
    va = ht @ Wa_w  # [B, E]
    ca = ht @ Wa_b  # [B]
    hs16 = hs.astype(np.float16)
    hsT16 = np.ascontiguousarray(hs16.transpose(0, 2, 1))  # [B, E, S]
    va16 = va.astype(np.float16)  # [B, E]

    # ---- sparse slotting: batches sorted by length desc, groups of NCORES ----
    L = state_len.astype(np.int64)
    tiles_b = np.maximum(np.ceil(L / 128).astype(np.int64), 1)
    order = np.argsort(-L, kind="stable")  # batch for (slot j, core c)
    slot_tiles = tuple(
        int(tiles_b[order[j * NCORES : (j + 1) * NCORES]].max()) for j in range(BL)
    )
    last_slot_tiles = slot_tiles

    if slot_tiles not in _NC_CACHE:
        _NC_CACHE[slot_tiles] = _build_bass(slot_tiles)
    nc = _NC_CACHE[slot_tiles]

    pos = np.arange(S, dtype=np.int64)
    valid = pos[None, :] < L[:, None]  # [B, S]
    cap = np.where(valid, 3.0e38, MASK_VAL).astype(np.float32)  # [B, S]
    consts = np.concatenate(
        [np.eye(128, dtype=np.float32), np.ones((128, 128), np.float32)], axis=1
    )

    in_maps = []
    for c in range(NCORES):
        m = {"consts": consts}
        va_dev = np.zeros((BL, 128, 2), np.float16)
        for j in range(BL):
            b = int(order[j * NCORES + c])
            nt = slot_tiles[j]
            sj = nt * 128
            hsT_dev = np.empty((128, 2 * sj), np.float16)
            hsT_dev[:, 0:sj] = hsT16[b, 0:128, 0:sj]
            hsT_dev[:, sj : 2 * sj] = hsT16[b, 128:256, 0:sj]
            m[f"hsT{j}"] = hsT_dev
            m[f"hsn{j}"] = np.ascontiguousarray(
                hs16[b, 0:sj].reshape(nt, 128, E).transpose(1, 0, 2)
            )
            aux_dev = np.empty((128, nt + 1), np.float32)
            aux_dev[:, 0:nt] = cap[b, 0:sj].reshape(nt, 128).T
            aux_dev[:, nt] = ca[b]
            m[f"aux{j}"] = aux_dev
            va_dev[j] = va16[b].reshape(2, 128).T
        m["va16"] = va_dev
        in_maps.append(m)

    last_in_maps = in_maps
    res = run_bass_kernel_spmd(nc, in_maps, core_ids=list(range(NCORES)), **run_kwargs)
    last_results = res

    # ---- unshard ----
    alpha = np.zeros((B, S), np.float32)
    u = np.zeros((B, E), np.float32)
    for c in range(NCORES):
        for j in range(BL):
            b = int(order[j * NCORES + c])
            nt = slot_tiles[j]
            sj = nt * 128
            a_dev = res.results[c][f"alpha{j}"]  # [128, nt]
            alpha[b, 0:sj] = a_dev.T.reshape(sj)
            u[b] = res.results[c]["u"][j].reshape(E)
    # state_len == 0: reference softmax is uniform over all S positions
    for b in np.nonzero(L == 0)[0]:
        alpha[b, :] = np.float32(1.0) / np.float32(S)
        u[b] = hs[b].mean(axis=0)
    context = (u @ Wc_w.T + Wc_b).astype(np.float32)
    return alpha, context


# revision 22
# speedup vs baseline: 1.0382x; 1.0382x over previous
"""Trainium2 Bass kernel for nn_Attention (sparse_attention).

Math (reference collapsed):
  va[b]    = ht[b] @ Wa_w          (host, tiny)
  ca[b]    = ht[b] . Wa_b          (host, tiny)
  energy   = leaky_relu(hs . va + ca), masked to -1e4 where s >= state_len
  alpha    = softmax(energy)                               (device)
  u[b]     = sum_s alpha[b,s] * hs[b,s,:]                  (device)
  context  = u @ Wc_w.T + Wc_b                             (host, tiny)

Sparsity: positions s >= state_len[b] have alpha exactly 0 (f32 exp
underflow) and never touch hs, so only ceil(state_len/128) s-tiles per
batch are shipped/computed. state_len is known when kernel() runs, so the
per-slot tile counts are baked into the Bass graph (rebuilt per distinct
state_len signature). Batches are assigned to cores in descending-length
groups of 8 so each slot's padding (to the slot max) is minimal.

Device strategy: pure data-parallel, 4 batch-slots/core x 8 cores, no
collectives. hs is shipped twice in fp16 (pre-transposed layout for the
energy matvec, partition-major natural layout for the alpha-weighted sum)
so both matmuls contract over the SBUF partition axis.
"""

import sys

sys.path.insert(0, "/opt/trn_rl_repo")

import numpy as np

import concourse.bass as bass
import concourse.tile as tile
from concourse import mybir
from concourse.bass_utils import run_bass_kernel_spmd
from concourse.vector_clock import ScopedClock

B, S, E = 32, 8192, 256
NCORES = 8
BL = B // NCORES  # batch slots per core
NEG_SLOPE = 0.2
MASK_VAL = -10000.0
AF = mybir.ActivationFunctionType
ALU = mybir.AluOpType

_PATCHED = False


def _patch_tile_drain():
    """Walrus in this env rejects >1 sem-wait on the kernel-exit Drain CTRL.

    Hoist the end-of-kernel waits onto single-wait sync nops instead.
    """
    global _PATCHED
    if _PATCHED:
        return
    _PATCHED = True

    def _drain_and_barrier(self, tick_clock, wait_clock):
        nc = self.nc
        carrier = nc.sync.nop(nofuse=True, hint="tile_exit_wait_carrier")
        wait_clock.add_sem_waits(
            carrier.ins, ScopedClock({None: tick_clock.global_clock})
        )
        si = carrier.ins.sync_info
        waits = list(si.on_wait) if si is not None else []
        if len(waits) > 1:
            carrier.ins.sync_info = mybir.SyncInfo(
                on_wait=[waits[0]], on_update=list(si.on_update)
            )
            for w in waits[1:]:
                n2 = nc.sync.nop(nofuse=True, hint="tile_exit_wait")
                n2.ins.sync_info = mybir.SyncInfo(on_wait=[w], on_update=[])
        nc.sync.drain(fusable=False)
        nc.all_engine_barrier()
        assert self.sems is not None
        popped = nc._tile_sem_poison_stack.pop()
        assert popped is self._sem_poison
        nc.clear_and_free_semaphores(list(self.sems.allocated().values()))
        nc.all_engine_barrier()

    tile.TileContext._drain_and_barrier = _drain_and_barrier


def _split_sync_waits(nc, max_waits=1):
    """Walrus in this env rejects >N sem-waits on a single instruction.

    Hoist excess waits onto same-engine NoOps placed immediately before the
    instruction — the engine sequencer executes in order, so waiting earlier
    on the same engine is semantically identical.
    """
    counter = 0
    for fn in nc.m.functions:
        for blk in fn.blocks:
            insts = list(blk.instructions)
            out = []
            changed = False
            for inst in insts:
                si = inst.sync_info
                waits = list(si.on_wait) if si is not None else []
                if len(waits) > max_waits:
                    keep = waits[:max_waits]
                    for w in waits[max_waits:]:
                        nop = mybir.InstNoOp(name=f"WSPLIT-{counter}")
                        counter += 1
                        nop.engine = inst.engine
                        nop.sync_info = mybir.SyncInfo(on_wait=[w], on_update=[])
                        out.append(nop)
                    inst.sync_info = mybir.SyncInfo(
                        on_wait=keep, on_update=list(si.on_update)
                    )
                    changed = True
                out.append(inst)
            if changed:
                blk.instructions = out


def _build_bass(slot_tiles, reps=1, dma_only=False):
    """slot_tiles: tuple of BL ints — s-tile count for each batch slot."""
    _patch_tile_drain()
    nc = bass.Bass("TRN2", target_bir_lowering=False, num_devices=NCORES)
    f16, f32 = mybir.dt.float16, mybir.dt.float32

    hsT_p, hsn_p, aux_p, alpha_p = [], [], [], []
    for j, nt in enumerate(slot_tiles):
        sj = nt * 128
        hsT_p.append(
            nc.declare_dram_parameter(f"hsT{j}", [128, 2 * sj], f16, isOutput=False)
        )
        # partition-major natural layout: hsn[p, t, :] = hs[t*128+p, :]
        hsn_p.append(
            nc.declare_dram_parameter(f"hsn{j}", [128, nt, E], f16, isOutput=False)
        )
        # aux[:, :nt] = mask cap (+BIG valid / -1e4 invalid), aux[:, nt] = ca
        aux_p.append(
            nc.declare_dram_parameter(f"aux{j}", [128, nt + 1], f32, isOutput=False)
        )
        alpha_p.append(
            nc.declare_dram_parameter(f"alpha{j}", [128, nt], f32, isOutput=True)
        )
    va16 = nc.declare_dram_parameter("va16", [BL, 128, 2], f16, isOutput=False)
    # consts[:, 0:128] = eye(128), consts[:, 128:256] = all-ones
    consts = nc.declare_dram_parameter("consts", [128, 256], f32, isOutput=False)
    u_o = nc.declare_dram_parameter("u", [BL, 1, E], f32, isOutput=True)

    max_nt = max(slot_tiles)
    with tile.TileContext(nc) as tc:
        with (
            tc.tile_pool(name="big", bufs=3) as big,
            tc.tile_pool(name="small", bufs=4) as small,
            tc.tile_pool(name="ones", bufs=1) as onesp,
            tc.tile_pool(name="pse", bufs=3, space="PSUM") as pse,
            tc.tile_pool(name="psu", bufs=2, space="PSUM") as psu,
            tc.tile_pool(name="pss", bufs=3, space="PSUM") as pss,
        ):
            cst = onesp.tile([128, 256], f32)
            nc.sync.dma_start(out=cst[:], in_=consts[:])
            ident = cst[:, 0:128]
            onescol = cst[:, 128:129]
            onesrow = cst[0:1, 128:256]

            def _slot_loads(j):
                nt = slot_tiles[j]
                sj = nt * 128
                # tags sized by max slot so all slots share pool slots
                hsT = big.tile([128, 2 * max_nt * 128], f16, tag="hsT")
                chunks = [(i, min(i + 16, nt)) for i in range(0, nt, 16)]
                for lo, hi in chunks:
                    for h in range(2):
                        nc.sync.dma_start(
                            out=hsT[:, h * sj + lo * 128 : h * sj + hi * 128],
                            in_=hsT_p[j][:, h * sj + lo * 128 : h * sj + hi * 128],
                        )
                hsn = big.tile([128, max_nt, E], f16, tag="hsn")
                for lo, hi in chunks:
                    nc.scalar.dma_start(
                        out=hsn[:, lo:hi, :], in_=hsn_p[j][:, lo:hi, :]
                    )
                va = small.tile([128, 2], f16, tag="va")
                nc.scalar.dma_start(out=va[:], in_=va16[j])
                ax = small.tile([128, max_nt + 1], f32, tag="aux")
                nc.scalar.dma_start(out=ax[:, 0 : nt + 1], in_=aux_p[j][:])
                return hsT, hsn, va, ax

            def _slot_body(j, loaded):
                nt = slot_tiles[j]
                sj = nt * 128
                hsT, hsn, va, ax = loaded

                if dma_only:
                    a32d = small.tile([128, max_nt], f32, tag="a32")
                    nc.vector.tensor_copy(a32d[:, 0:nt], ax[:, 0:nt])
                    nc.sync.dma_start(out=alpha_p[j][:], in_=a32d[:, 0:nt])
                    return

                # ---- energy: e[t*128+p] = sum_e hs[s,e]*va[e]  (PE) ----
                pe = pse.tile([128, max_nt], f32, tag="pe")
                for t in range(nt):
                    for h in range(2):
                        nc.tensor.matmul(
                            out=pe[:, t : t + 1],
                            lhsT=hsT[:, h * sj + t * 128 : h * sj + (t + 1) * 128],
                            rhs=va[:, h : h + 1],
                            start=(h == 0),
                            stop=(h == 1),
                        )

                # ---- epilogue: lrelu(energy + ca) ----
                esb = small.tile([128, max_nt], f32, tag="esb")
                nc.scalar.activation(
                    out=esb[:, 0:nt],
                    in_=pe[:, 0:nt],
                    func=AF.Lrelu,
                    bias=ax[:, nt : nt + 1],
                    scale=1.0,
                    alpha=NEG_SLOPE,
                )
                # ---- mask via min(e, cap), then row-max ----
                em = small.tile([128, max_nt], f32, tag="em")
                nc.vector.tensor_tensor(
                    out=em[:, 0:nt], in0=esb[:, 0:nt], in1=ax[:, 0:nt], op=ALU.min
                )
                mrow = small.tile([128, 1], f32, tag="mrow")
                nc.vector.reduce_max(
                    out=mrow[:], in_=em[:, 0:nt], axis=mybir.AxisListType.X
                )
                # ---- global max: transpose mrow via identity-matmul, reduce,
                #      broadcast back via ones-matmul, negate ----
                psT = pss.tile([1, 128], f32, tag="ps_small")
                nc.tensor.matmul(
                    out=psT[:], lhsT=mrow[:], rhs=ident, start=True, stop=True
                )
                msc = small.tile([1, 1], f32, tag="msc")
                nc.vector.reduce_max(out=msc[:], in_=psT[:], axis=mybir.AxisListType.X)
                psB = pss.tile([128, 1], f32, tag="ps_small")
                nc.tensor.matmul(
                    out=psB[:], lhsT=onesrow, rhs=msc[:], start=True, stop=True
                )
                negm = small.tile([128, 1], f32, tag="negm")
                nc.vector.tensor_scalar_mul(negm[:], psB[:], -1.0)
                # ---- exp (+ row sums) ----
                p16 = small.tile([128, max_nt], f16, tag="p16")
                zrow = small.tile([128, 1], f32, tag="zrow")
                nc.scalar.activation(
                    out=p16[:, 0:nt],
                    in_=em[:, 0:nt],
                    func=AF.Exp,
                    bias=negm[:],
                    scale=1.0,
                    accum_out=zrow[:],
                )
                # ---- global sum via ones-matmul, reciprocal, broadcast ----
                psZ = pss.tile([1, 1], f32, tag="ps_small")
                nc.tensor.matmul(
                    out=psZ[:], lhsT=zrow[:], rhs=onescol, start=True, stop=True
                )
                rz1 = small.tile([1, 1], f32, tag="rz1")
                nc.vector.reciprocal(rz1[:], psZ[:])
                psR = pss.tile([128, 1], f32, tag="ps_small")
                nc.tensor.matmul(
                    out=psR[:], lhsT=onesrow, rhs=rz1[:], start=True, stop=True
                )
                rz = small.tile([128, 1], f32, tag="rz")
                nc.vector.tensor_copy(rz[:], psR[:])

                # ---- alpha output ----
                a32 = small.tile([128, max_nt], f32, tag="a32")
                nc.vector.tensor_scalar_mul(a32[:, 0:nt], p16[:, 0:nt], rz[:])
                nc.sync.dma_start(out=alpha_p[j][:], in_=a32[:, 0:nt])

                # ---- u = sum_s p[s]*hs[s,:] (PE, accumulated), then /z ----
                pu = psu.tile([1, E], f32, tag="pu")
                for t in range(nt):
                    nc.tensor.matmul(
                        out=pu[:],
                        lhsT=p16[:, t : t + 1],
                        rhs=hsn[:, t, :],
                        start=(t == 0),
                        stop=(t == nt - 1),
                    )
                usb = small.tile([1, E], f32, tag="usb")
                nc.scalar.activation(
                    out=usb[:], in_=pu[:], func=AF.Copy, scale=rz[0:1, :]
                )
                nc.sync.dma_start(out=u_o[j], in_=usb[:])

            def _trace_all():
                # loads for slot j+1 trace before slot j's compute so the
                # scheduler prioritizes keeping the DMA queues full
                loaded = {0: _slot_loads(0), 1: _slot_loads(1)}
                for j in range(BL):
                    nxt = j + 2
                    if nxt < BL:
                        loaded[nxt] = _slot_loads(nxt)
                    _slot_body(j, loaded.pop(j))

            if reps == 1:
                _trace_all()
            else:
                with tc.For_i(0, reps, 1):
                    _trace_all()
    _split_sync_waits(nc)
    return nc


_NC_CACHE = {}
last_results = None
last_in_maps = None
last_slot_tiles = None


def kernel(hs, ht, state_len, Wa_w, Wa_b, Wc_w, Wc_b, **run_kwargs):
    global last_results, last_in_maps, last_slot_tiles
    hs = np.asarray(hs, np.float32)
    ht = np.asarray(ht, np.float32)
    state_len = np.asarray(state_len, np.int32)
    Wa_w = np.asarray(Wa_w, np.float32)
    Wa_b = np.asarray(Wa_b, np.float32)
    Wc_w = np.asarray(Wc_w, np.float32)
    Wc_b = np.asarray(Wc_b, np.float32)

    # host-side tiny precomputes
    va = ht @ Wa_w  # [B, E]
    ca = ht @ Wa_b  # [B]
    hs16 = hs.astype(np.float16)
    hsT16 = np.ascontiguousarray(hs16.transpose(0, 2, 1))  # [B, E, S]
    va16 = va.astype(np.float16)  # [B, E]

    # ---- sparse slotting: batches sorted by length desc, groups of NCORES ----
    L = state_len.astype(np.int64)
    tiles_b = np.maximum(np.ceil(L / 128).astype(np.int64), 1)
    order = np.argsort(-L, kind="stable")  # batch for (slot j, core c)
    slot_tiles = tuple(
        int(tiles_b[order[j * NCORES : (j + 1) * NCORES]].max()) for j in range(BL)
    )
    last_slot_tiles = slot_tiles

    if slot_tiles not in _NC_CACHE:
        _NC_CACHE[slot_tiles] = _build_bass(slot_tiles)
    nc = _NC_CACHE[slot_tiles]

    pos = np.arange(S, dtype=np.int64)
    valid = pos[None, :] < L[:, None]  # [B, S]
    cap = np.where(valid, 3.0e38, MASK_VAL).astype(np.float32)  # [B, S]
    consts = np.concatenate(
        [np.eye(128, dtype=np.float32), np.ones((128, 128), np.float32)], axis=1
    )

    in_maps = []
    for c in range(NCORES):
        m = {"consts": consts}
        va_dev = np.zeros((BL, 128, 2), np.float16)
        for j in range(BL):
            b = int(order[j * NCORES + c])
            nt = slot_tiles[j]
            sj = nt * 128
            hsT_dev = np.empty((128, 2 * sj), np.float16)
            hsT_dev[:, 0:sj] = hsT16[b, 0:128, 0:sj]
            hsT_dev[:, sj : 2 * sj] = hsT16[b, 128:256, 0:sj]
            m[f"hsT{j}"] = hsT_dev
            m[f"hsn{j}"] = np.ascontiguousarray(
                hs16[b, 0:sj].reshape(nt, 128, E).transpose(1, 0, 2)
            )
            aux_dev = np.empty((128, nt + 1), np.float32)
            aux_dev[:, 0:nt] = cap[b, 0:sj].reshape(nt, 128).T
            aux_dev[:, nt] = ca[b]
            m[f"aux{j}"] = aux_dev
            va_dev[j] = va16[b].reshape(2, 128).T
        m["va16"] = va_dev
        in_maps.append(m)

    last_in_maps = in_maps
    res = run_bass_kernel_spmd(nc, in_maps, core_ids=list(range(NCORES)), **run_kwargs)
    last_results = res

    # ---- unshard ----
    alpha = np.zeros((B, S), np.float32)
    u = np.zeros((B, E), np.float32)
    for c in range(NCORES):
        for j in range(BL):
            b = int(order[j * NCORES + c])
            nt = slot_tiles[j]
            sj = nt * 128
            a_dev = res.results[c][f"alpha{j}"]  # [128, nt]
            alpha[b, 0:sj] = a_dev.T.reshape(sj)
            u[b] = res.results[c]["u"][j].reshape(E)
    # state_len == 0: reference softmax is uniform over all S positions
    for b in np.nonzero(L == 0)[0]:
        alpha[b, :] = np.float32(1.0) / np.float32(S)
        u[b] = hs[b].mean(axis=0)
    context = (u @ Wc_w.T + Wc_b).astype(np.float32)
    return alpha, context
